# revision 7
# baseline (speedup 1.0000x reference)
"""Trainium2 Bass kernel for a transformer decoder layer (self-attn + cross-attn + FFN).

Sharding: 8 cores = 4 batches x 2 sequence-halves. Each core computes its
1024 query tokens end-to-end (no collectives). K/V are computed from the
full 2048-token sequence on every core; causal masking is data-driven
(per-core global index vectors) so the program is SPMD-uniform.

Layouts: activations for matmul are kept transposed ([d, tokens], d on
partitions) so projections, scores (K^T Q) and attn@V (E^T V) all contract
along partitions with zero on-chip transposes except the two residual-stream
transposes (y1, y2) which run on the PE. Softmax runs without max-subtraction
(scores are bounded ~|2.6| for this problem's scale); the denominator comes
from an extra all-ones column appended to V.
"""

from contextlib import ExitStack

import numpy as np

import concourse.bass as bass
import concourse.mybir as mybir
import concourse.tile as tile
from concourse import bacc
from concourse.bass_utils import run_bass_kernel_spmd
from concourse.masks import make_identity

f32 = mybir.dt.float32
f16 = mybir.dt.float16

P = 128
D = 1024          # d_model
S = 2048          # kv sequence length
NQ = 1024         # query tokens per core
DFF = 4096
DTI = D // P      # 8 d-model partition tiles
KTI = S // P      # 16 kv token tiles
QTI = NQ // P     # 8 query tiles
FTI = DFF // P    # 32 d_ff tiles
NCH = NQ // 512   # 2 query chunks of 512
ACT = mybir.ActivationFunctionType
ALU = mybir.AluOpType
N_CORES = 8
SCALE = 1.0 / 32.0  # 1/sqrt(D)


def _self_visible(t, c):
    """Self-attn block (k-tile t, q-chunk c) possibly visible on some core?

    KV is reordered per-core to [own half | other half], so tiles t<8 are the
    core's own half: standard causal pattern, identical across cores. Tiles
    t>=8 are the other half: all-visible or all-invisible depending on the
    core, so they stay in the program and the data mask decides.
    """
    if t < KTI // 2:
        return t * P < (c + 1) * 512
    return True


def _self_needs_mask(t, c):
    if t < KTI // 2:
        # own half: fully-visible blocks need no mask; diagonal blocks do
        return 4 * c <= t <= 4 * c + 3
    return True


def build_nc():
    nc = bacc.Bacc("TRN2", target_bir_lowering=False, debug=False,
                   num_devices=N_CORES)

    def dp(name, shape, dt, out=False):
        return nc.declare_dram_parameter(name, shape, dt, isOutput=out)

    ykvT_d = dp("ykvT", [D, S], f16)
    zT_d = dp("zT", [D, S], f16)
    yres_d = dp("yres", [NQ, D], f16)
    qg_d = dp("qg", [NQ], f32)
    kg_d = dp("kg", [S], f32)
    w_d = {n: dp(n, [D, D], f16)
           for n in ["wq1", "wk1", "wv1", "wq2", "wk2", "wv2"]}
    wf1_d = dp("wf1", [D, DFF], f16)
    wf2_d = dp("wf2", [DFF, D], f16)
    bf1_d = dp("bf1", [P, FTI], f32)
    vec_d = {n: dp(n, [D], f32)
             for n in ["bf2", "g1", "be1", "g2", "be2", "g3", "be3"]}
    out_d = dp("out", [NQ, D], f32, out=True)

    def bc(ap):  # broadcast a [n] dram vector across 128 partitions
        return bass.AP(tensor=ap.tensor, offset=ap.offset,
                       ap=[[0, P]] + [list(x) for x in ap.ap])

    with tile.TileContext(nc) as tc, ExitStack() as top:
        const = top.enter_context(tc.tile_pool(name="const", bufs=1))
        ident = const.tile([P, P], f16, name="ident", tag="ident")
        make_identity(nc, ident)
        kidx = const.tile([P, KTI], f32, name="kidx", tag="kidx")
        nc.sync.dma_start(out=kidx, in_=kg_d.ap().rearrange("(n p) -> p n", p=P))
        qgb = const.tile([P, NQ], f32, name="qgb", tag="qgb")
        nc.sync.dma_start(out=qgb, in_=bc(qg_d.ap()))
        eps = const.tile([P, 1], f32, name="eps", tag="eps")
        nc.vector.memset(eps, 1e-5)
        bf1_sb = const.tile([P, FTI], f32, name="bf1_sb", tag="bf1")
        nc.sync.dma_start(out=bf1_sb, in_=bf1_d.ap())

        def load_vec_bcast(pool, name):
            t = pool.tile([P, D], f32, name=f"{name}_sb", tag=f"vb_{name}")
            nc.sync.dma_start(out=t, in_=bc(vec_d[name].ap()))
            return t

        def load_weight(pool, dram, wname):
            tiles = []
            for j in range(DTI):
                t = pool.tile([P, D], f16, name=f"{wname}{j}", tag=f"w{j}")
                nc.sync.dma_start(out=t, in_=dram.ap()[j * P:(j + 1) * P, :])
                tiles.append(t)
            return tiles

        def layer_norm(lnp, x, gb, bb, out):
            """out = (x - mean) * rsqrt(var + eps) * gb + bb, per partition row."""
            stats = lnp.tile([P, 2, 6], f32, name="stats", tag="stats")
            nc.vector.bn_stats(out=stats[:, 0, :], in_=x[:, 0:512])
            nc.vector.bn_stats(out=stats[:, 1, :], in_=x[:, 512:1024])
            mv = lnp.tile([P, 2], f32, name="mv", tag="mv")
            nc.vector.bn_aggr(out=mv, in_=stats)
            std = lnp.tile([P, 1], f32, name="std", tag="std")
            nc.scalar.activation(out=std, in_=mv[:, 1:2], func=ACT.Sqrt,
                                 bias=eps, scale=1.0)
            rstd = lnp.tile([P, 1], f32, name="rstd", tag="rstd")
            nc.vector.reciprocal(rstd, std)
            tmp = lnp.tile([P, D], f32, name="lntmp", tag="lntmp", bufs=2)
            nc.vector.tensor_scalar(out=tmp, in0=x, scalar1=mv[:, 0:1],
                                    scalar2=rstd, op0=ALU.subtract,
                                    op1=ALU.mult)
            nc.vector.tensor_mul(out=tmp, in0=tmp, in1=gb)
            nc.vector.tensor_add(out=out, in0=tmp, in1=bb)

        def project_qT(psum, wtiles, src, qT, nchunks):
            # qT[i][:, c*512:...] = (W.T @ src), contraction over d_in tiles
            for i in range(DTI):
                for c in range(nchunks):
                    ps = psum.tile([P, 512], f32, name="ps_proj", tag="ps_proj")
                    for j in range(DTI):
                        nc.tensor.matmul(ps, lhsT=wtiles[j][:, i * P:(i + 1) * P],
                                         rhs=src[j][:, c * 512:(c + 1) * 512],
                                         start=(j == 0), stop=(j == DTI - 1))
                    nc.scalar.copy(out=qT[i][:, c * 512:(c + 1) * 512], in_=ps)

        def project_v(psum, src, wtiles, v):
            # v[t][:, n*512:...] = src_tile.T @ W  (natural [token, d] layout)
            for t in range(KTI):
                for n in range(2):
                    ps = psum.tile([P, 512], f32, name="ps_v", tag="ps_proj")
                    for j in range(DTI):
                        nc.tensor.matmul(ps, lhsT=src[j][:, t * P:(t + 1) * P],
                                         rhs=wtiles[j][:, n * 512:(n + 1) * 512],
                                         start=(j == 0), stop=(j == DTI - 1))
                    nc.scalar.copy(out=v[t][:, n * 512:(n + 1) * 512], in_=ps)
                nc.vector.memset(v[t][:, 1024:1025], 1.0)

        def attention(stk, qT, kT, v, resid, gb, bb, y_out, masked):
            """scoresT = K^T Q per block -> exp -> (mask) -> out = E^T [V|1]."""
            psum_s = stk.enter_context(tc.tile_pool(name="psum_s", bufs=2,
                                                    space="PSUM"))
            psum_o = stk.enter_context(tc.tile_pool(name="psum_o", bufs=2,
                                                    space="PSUM"))
            expp = stk.enter_context(tc.tile_pool(name="expp", bufs=2))
            maskp = stk.enter_context(tc.tile_pool(name="maskp", bufs=2))
            lnp = stk.enter_context(tc.tile_pool(name="lnp", bufs=4))
            for c in range(NCH):
                qsl = slice(c * 512, (c + 1) * 512)
                vis = [t for t in range(KTI)
                       if not masked or _self_visible(t, c)]
                etiles = {}
                for t in vis:
                    ps = psum_s.tile([P, 512], f32, name="ps_s", tag="ps_s")
                    for j in range(DTI):
                        nc.tensor.matmul(ps, lhsT=kT[j][:, t * P:(t + 1) * P],
                                         rhs=qT[j][:, qsl],
                                         start=(j == 0), stop=(j == DTI - 1))
                    e = expp.tile([P, 512], f16, name="e", tag=f"e{t}")
                    nc.scalar.activation(out=e, in_=ps, func=ACT.Exp,
                                         scale=SCALE)
                    if masked and _self_needs_mask(t, c):
                        m = maskp.tile([P, 512], f16, name="m", tag="mask")
                        nc.vector.tensor_scalar(out=m, in0=qgb[:, qsl],
                                                scalar1=kidx[:, t:t + 1],
                                                scalar2=None, op0=ALU.is_ge)
                        nc.vector.tensor_mul(out=e, in0=e, in1=m)
                    etiles[t] = e
                for u4 in range(4):
                    u = c * 4 + u4
                    po = psum_o.tile([P, 1536], f32, name="po", tag="po")
                    for idx, t in enumerate(vis):
                        st, sp = idx == 0, idx == len(vis) - 1
                        lhsT = etiles[t][:, u4 * P:(u4 + 1) * P]
                        nc.tensor.matmul(po[:, 0:512], lhsT=lhsT,
                                         rhs=v[t][:, 0:512], start=st, stop=sp)
                        nc.tensor.matmul(po[:, 512:1024], lhsT=lhsT,
                                         rhs=v[t][:, 512:1024], start=st,
                                         stop=sp)
                        nc.tensor.matmul(po[:, 1024:1025], lhsT=lhsT,
                                         rhs=v[t][:, 1024:1025], start=st,
                                         stop=sp)
                    rec = lnp.tile([P, 1], f32, name="rec", tag="rec")
                    nc.vector.reciprocal(rec, po[:, 1024:1025])
                    xr = lnp.tile([P, D], f32, name="xr", tag="xr",
                                  bufs=2)
                    nc.vector.tensor_scalar(out=xr, in0=po[:, 0:1024],
                                            scalar1=rec, scalar2=None,
                                            op0=ALU.mult)
                    nc.vector.tensor_add(out=xr, in0=xr, in1=resid[u])
                    layer_norm(lnp, xr, gb, bb, y_out[u])

        def transpose_qd(stk, y_h, y_T):
            # y_h[u]: [128q, 1024d] f16  ->  y_T[i]: [128d, 1024q] f16
            psum_t = stk.enter_context(tc.tile_pool(name="psum_t", bufs=2,
                                                    space="PSUM"))
            for i in range(DTI):
                for u in range(QTI):
                    pt = psum_t.tile([P, P], f16, name="pt", tag="pt")
                    nc.tensor.transpose(pt, in_=y_h[u][:, i * P:(i + 1) * P],
                                        identity=ident)
                    nc.scalar.copy(out=y_T[i][:, u * P:(u + 1) * P], in_=pt)

        # ---------------- pools with cross-stage lifetimes -----------------
        qkvp = tc.alloc_tile_pool(name="qkvp", bufs=1)   # qT/kT/v: A..B
        y1p = tc.alloc_tile_pool(name="y1p", bufs=1, side="right")
        y1h = [y1p.tile([P, D], f16, name=f"y1h{u}", tag=f"y1h{u}")
               for u in range(QTI)]

        # ============ stage A: self-attn QKV projections ============
        with ExitStack() as stA:
            kvp = stA.enter_context(tc.tile_pool(name="kvp", bufs=1))
            wp = stA.enter_context(tc.tile_pool(name="wp", bufs=2))
            psum_a = stA.enter_context(tc.tile_pool(name="psum_a", bufs=4,
                                                    space="PSUM"))
            ykv = [kvp.tile([P, S], f16, name=f"ykv{j}", tag=f"kv{j}")
                   for j in range(DTI)]
            for j in range(DTI):
                nc.sync.dma_start(out=ykv[j],
                                  in_=ykvT_d.ap()[j * P:(j + 1) * P, :])
            qT = [qkvp.tile([P, NQ], f16, name=f"qT{i}", tag=f"qT{i}")
                  for i in range(DTI)]
            kT = [qkvp.tile([P, S], f16, name=f"kT{i}", tag=f"kT{i}")
                  for i in range(DTI)]
            v = [qkvp.tile([P, 1025], f16, name=f"v{t}", tag=f"v{t}")
                 for t in range(KTI)]

            wq = load_weight(wp, w_d["wq1"], "wq1")
            project_qT(psum_a, wq, ykv, qT, NCH)
            wk = load_weight(wp, w_d["wk1"], "wk1")
            # K over all kv chunks (4 x 512)
            for i in range(DTI):
                for ck in range(S // 512):
                    ps = psum_a.tile([P, 512], f32, name="ps_k", tag="ps_proj")
                    for j in range(DTI):
                        nc.tensor.matmul(ps, lhsT=wk[j][:, i * P:(i + 1) * P],
                                         rhs=ykv[j][:, ck * 512:(ck + 1) * 512],
                                         start=(j == 0), stop=(j == DTI - 1))
                    nc.scalar.copy(out=kT[i][:, ck * 512:(ck + 1) * 512], in_=ps)
            wv = load_weight(wp, w_d["wv1"], "wv1")
            project_v(psum_a, ykv, wv, v)

        # ============ stage B: self-attention + LN1 ============
        with ExitStack() as stB:
            resp = stB.enter_context(tc.tile_pool(name="resp", bufs=1))
            gbp = stB.enter_context(tc.tile_pool(name="gbp1", bufs=1))
            yres = [resp.tile([P, D], f16, name=f"yres{u}", tag=f"yres{u}")
                    for u in range(QTI)]
            for u in range(QTI):
                nc.sync.dma_start(out=yres[u],
                                  in_=yres_d.ap()[u * P:(u + 1) * P, :])
            g1b = load_vec_bcast(gbp, "g1")
            be1b = load_vec_bcast(gbp, "be1")
            attention(stB, qT, kT, v, yres, g1b, be1b, y1h, masked=True)
        qkvp.release()

        # transpose y1 -> y1T for cross-attn Q projection
        y1Tp = tc.alloc_tile_pool(name="y1Tp", bufs=1)   # left: B2..C1
        y1T = [y1Tp.tile([P, NQ], f16, name=f"y1T{i}", tag=f"y1T{i}")
               for i in range(DTI)]
        with ExitStack() as stB2:
            transpose_qd(stB2, y1h, y1T)

        # ============ stage C: cross-attention + LN2 ============
        qkv2p = tc.alloc_tile_pool(name="qkv2p", bufs=1, side="right")
        qT2 = [qkv2p.tile([P, NQ], f16, name=f"qT2{i}", tag=f"qT2{i}")
               for i in range(DTI)]
        kT2 = [qkv2p.tile([P, S], f16, name=f"kT2{i}", tag=f"kT2{i}")
               for i in range(DTI)]
        v2 = [qkv2p.tile([P, 1025], f16, name=f"v2{t}", tag=f"v2{t}")
              for t in range(KTI)]
        with ExitStack() as stC1:
            zp = stC1.enter_context(tc.tile_pool(name="zp", bufs=1))
            wp2 = stC1.enter_context(tc.tile_pool(name="wp2", bufs=2))
            psum_c = stC1.enter_context(tc.tile_pool(name="psum_c", bufs=4,
                                                     space="PSUM"))
            zT = [zp.tile([P, S], f16, name=f"zT{j}", tag=f"z{j}")
                  for j in range(DTI)]
            for j in range(DTI):
                nc.sync.dma_start(out=zT[j],
                                  in_=zT_d.ap()[j * P:(j + 1) * P, :])
            wq2 = load_weight(wp2, w_d["wq2"], "wq2")
            project_qT(psum_c, wq2, y1T, qT2, NCH)
            wk2 = load_weight(wp2, w_d["wk2"], "wk2")
            for i in range(DTI):
                for ck in range(S // 512):
                    ps = psum_c.tile([P, 512], f32, name="ps_k2",
                                     tag="ps_proj")
                    for j in range(DTI):
                        nc.tensor.matmul(
                            ps, lhsT=wk2[j][:, i * P:(i + 1) * P],
                            rhs=zT[j][:, ck * 512:(ck + 1) * 512],
                            start=(j == 0), stop=(j == DTI - 1))
                    nc.scalar.copy(out=kT2[i][:, ck * 512:(ck + 1) * 512],
                                   in_=ps)
            wv2 = load_weight(wp2, w_d["wv2"], "wv2")
            project_v(psum_c, zT, wv2, v2)
        y1Tp.release()

        y2p = tc.alloc_tile_pool(name="y2p", bufs=1)      # y2h: C2..D
        y2h = [y2p.tile([P, D], f16, name=f"y2h{u}", tag=f"y2h{u}")
               for u in range(QTI)]
        with ExitStack() as stC2:
            gbp2 = stC2.enter_context(tc.tile_pool(name="gbp2", bufs=1))
            g2b = load_vec_bcast(gbp2, "g2")
            be2b = load_vec_bcast(gbp2, "be2")
            attention(stC2, qT2, kT2, v2, y1h, g2b, be2b, y2h,
                      masked=False)
        qkv2p.release()
        y1p.release()

        y2Tp = tc.alloc_tile_pool(name="y2Tp", bufs=1)    # y2T: C3..D
        y2T = [y2Tp.tile([P, NQ], f16, name=f"y2T{i}", tag=f"y2T{i}")
               for i in range(DTI)]
        with ExitStack() as stC3:
            transpose_qd(stC3, y2h, y2T)

        # ============ stage D: FFN + LN3 + output ============
        with ExitStack() as stD:
            wf2p = stD.enter_context(tc.tile_pool(name="wf2p", bufs=1))
            wf1p = stD.enter_context(tc.tile_pool(name="wf1p", bufs=3))
            htp = stD.enter_context(tc.tile_pool(name="htp", bufs=1))
            gbp3 = stD.enter_context(tc.tile_pool(name="gbp3", bufs=1))
            outp = stD.enter_context(tc.tile_pool(name="outp", bufs=2))
            ln3p = stD.enter_context(tc.tile_pool(name="ln3p", bufs=4))
            psum_h = stD.enter_context(tc.tile_pool(name="psum_h", bufs=4,
                                                    space="PSUM"))
            psum_f = stD.enter_context(tc.tile_pool(name="psum_f", bufs=2,
                                                    space="PSUM"))
            wf2 = [wf2p.tile([P, D], f16, name=f"wf2_{s}", tag=f"wf2_{s}")
                   for s in range(FTI)]
            for s in range(FTI):
                nc.sync.dma_start(out=wf2[s],
                                  in_=wf2_d.ap()[s * P:(s + 1) * P, :])
            g3b = load_vec_bcast(gbp3, "g3")
            be3b = load_vec_bcast(gbp3, "be3")
            bf2b = load_vec_bcast(gbp3, "bf2")

            for c in range(NCH):
                qsl = slice(c * 512, (c + 1) * 512)
                hts = []
                for s in range(FTI):
                    wt = wf1p.tile([P, DTI, P], f16, name="wf1s", tag="wf1s")
                    nc.sync.dma_start(
                        out=wt,
                        in_=wf1_d.ap()[:, s * P:(s + 1) * P].rearrange(
                            "(n p) m -> p n m", p=P))
                    ph = psum_h.tile([P, 512], f32, name="ph", tag="ph")
                    for j in range(DTI):
                        nc.tensor.matmul(ph, lhsT=wt[:, j, :],
                                         rhs=y2T[j][:, qsl],
                                         start=(j == 0), stop=(j == DTI - 1))
                    ht = htp.tile([P, 512], f16, name="ht", tag=f"ht{s}")
                    nc.scalar.activation(out=ht, in_=ph, func=ACT.Relu,
                                         bias=bf1_sb[:, s:s + 1], scale=1.0)
                    hts.append(ht)
                for u4 in range(4):
                    u = c * 4 + u4
                    pf = psum_f.tile([P, D], f32, name="pf", tag="pf")
                    for n in range(2):
                        for s in range(FTI):
                            nc.tensor.matmul(
                                pf[:, n * 512:(n + 1) * 512],
                                lhsT=hts[s][:, u4 * P:(u4 + 1) * P],
                                rhs=wf2[s][:, n * 512:(n + 1) * 512],
                                start=(s == 0), stop=(s == FTI - 1))
                    xr = ln3p.tile([P, D], f32, name="xr3", tag="xr3", bufs=2)
                    nc.vector.tensor_add(out=xr, in0=pf, in1=bf2b)
                    nc.vector.tensor_add(out=xr, in0=xr, in1=y2h[u])
                    y3 = outp.tile([P, D], f32, name="y3", tag="y3")
                    layer_norm(ln3p, xr, g3b, be3b, y3)
                    nc.sync.dma_start(out=out_d.ap()[u * P:(u + 1) * P, :],
                                      in_=y3)
        y2Tp.release()
        y2p.release()

    nc.compile()
    return nc


_CACHE = {}


def _get_nc():
    if "nc" not in _CACHE:
        _CACHE["nc"] = build_nc()
    return _CACHE["nc"]


def _prep_core(c, y, Z, shared):
    b, h = c // 2, c % 2
    own = np.arange(h * 1024, (h + 1) * 1024)
    oth = np.arange(1024, 2048) if h == 0 else np.arange(0, 1024)
    order = np.concatenate([own, oth])
    yb = y[b]
    m = {
        "ykvT": np.ascontiguousarray(yb[order].T).astype(np.float16),
        "zT": np.ascontiguousarray(Z[b].T).astype(np.float16),
        "yres": yb[own].astype(np.float16),
        "qg": own.astype(np.float32),
        "kg": order.astype(np.float32),
    }
    m.update(shared)
    return m


def kernel(**inputs):
    inp = {k: np.asarray(v) for k, v in inputs.items()}
    y = inp["y"].astype(np.float32)
    Z = inp["Z"].astype(np.float32)
    shared = {
        "wq1": inp["WQ1"].astype(np.float16),
        "wk1": inp["WK1"].astype(np.float16),
        "wv1": inp["WV1"].astype(np.float16),
        "wq2": inp["WQ2"].astype(np.float16),
        "wk2": inp["WK2"].astype(np.float16),
        "wv2": inp["WV2"].astype(np.float16),
        "wf1": inp["W_ff1"].astype(np.float16),
        "wf2": inp["W_ff2"].astype(np.float16),
        "bf1": np.ascontiguousarray(
            inp["b_ff1"].astype(np.float32).reshape(FTI, P).T),
        "bf2": inp["b_ff2"].astype(np.float32),
        "g1": inp["g1"].astype(np.float32),
        "be1": inp["be1"].astype(np.float32),
        "g2": inp["g2"].astype(np.float32),
        "be2": inp["be2"].astype(np.float32),
        "g3": inp["g3"].astype(np.float32),
        "be3": inp["be3"].astype(np.float32),
    }
    in_maps = [_prep_core(c, y, Z, shared) for c in range(N_CORES)]
    res = run_bass_kernel_spmd(_get_nc(), in_maps, list(range(N_CORES)))
    out = np.zeros((4, 2048, 1024), np.float32)
    for c in range(N_CORES):
        b, h = c // 2, c % 2
        out[b, h * 1024:(h + 1) * 1024] = res.results[c]["out"]
    return out
